# revision 3
# baseline (speedup 1.0000x reference)
"""Contrastive-learning NCE loss on 8 trn2 NeuronCores (Bass/Tile).

Problem (hardcoded shapes): B=8, L=1024, D_in=512, D_feat=256, N=B*L=8192.
  emb_k = relu(feature_k @ W + b)                     [B, L, Df]
  positive = <e1,e2> + banded_diag_mean terms         [N]
  negative = logsumexp(e1 @ e2.T, axis=-1) - log(N)   [N]
  loss = mean(-positive + negative)

Sharding: token dim N split across 8 cores = one batch row each (L == N/8).
Each core computes its [1024, 8192] slab of the similarity matrix against the
full emb_2 (recomputed locally from full feature2). The host rotates feature2
per core so the core's own batch always sits at columns 0:1023 -> the SPMD
program is core-index free.

v2 (fp8): all matmuls run fp8e4m3 with MatmulPerfMode.DoubleRow (K=256 per
instruction, ~1.5-2x bf16 rate). Host pre-scales W,b by 16 so W entries leave
the fp8 subnormal range; embeddings are stored at 16x scale (fp8 for matmuls,
bf16 copies of the own-batch columns for the banded terms). All dot products
come out at 256x scale: the EXP activation applies scale=1/256, the diag bias
is multiplied by -1/256 on device, and the host divides pos_* by 256.

The kernel is ACT-bound: exp over the [1024, 8192] slab costs ~(N+352)/1.2 ns
per activation ~= 70us/core. The schedule keeps ACT busy from ~t=10us: sim
column group 0 (own chunk, projected first) streams EXPs while the remaining
f2 projection chunks interleave on the PE; the banded-term DVE work (boxsums,
products) is emitted after the last projection epilogue so the in-order DVE
queue never blocks an epilogue the PE is waiting on; the banded row-sum
matmuls run on the PE after the last sim matmul, overlapping the final EXPs.

logsumexp per row with shift C = diag = <e1_m, e2_m> (exact for any C; host
finishes lse = C + log(S), and C cancels against the main positive term).
"""

import numpy as np
import ml_dtypes
from contextlib import ExitStack

import concourse.bass as bass
import concourse.tile as tile
from concourse import bacc, mybir
from concourse import bass_utils

dt = mybir.dt
AF = mybir.ActivationFunctionType
ALU = mybir.AluOpType
DR = mybir.MatmulPerfMode.DoubleRow

N_CORES = 8
B, L, DIN, DF = 8, 1024, 512, 256
N = B * L
KO = DIN // 128     # 4 k-tiles of the projection contraction
PAD = 4             # box-filter padding (max supported positive_range)
CW = 2048           # column group width: f2 proj chunks AND sim/EXP tiles
NCG = N // CW       # 4 column groups
SC = 16.0           # host pre-scale on W, b; emb stored at 16x

_module_cache = {}


def _box_terms(w: int):
    """Decompose window width w (odd, <= 2*PAD+1) into power-of-2 segments:
    returns [(pow, offset), ...] s.t. window = concat of segments."""
    terms, off = [], 0
    for p in (8, 4, 2, 1):
        if w >= p:
            terms.append((p, off))
            off += p
            w -= p
    assert w == 0
    return terms


def _build(r_self: int, r_tgt: int):
    nc = bacc.Bacc("TRN2", target_bir_lowering=False, debug=False, num_devices=N_CORES)

    f1t = nc.dram_tensor("f1t", [DIN, L], dt.float8e4, kind="ExternalInput").ap()
    f2t = nc.dram_tensor("f2t", [DIN, N], dt.float8e4, kind="ExternalInput").ap()
    w_in = nc.dram_tensor("w_in", [DIN, DF], dt.float8e4, kind="ExternalInput").ap()
    b_in = nc.dram_tensor("b_in", [DF], dt.float32, kind="ExternalInput").ap()

    pos_main = nc.dram_tensor("pos_main", [L], dt.float32, kind="ExternalOutput").ap()
    pos_self = nc.dram_tensor("pos_self", [L], dt.float32, kind="ExternalOutput").ap()
    pos_tgt = nc.dram_tensor("pos_tgt", [L], dt.float32, kind="ExternalOutput").ap()
    s_out = nc.dram_tensor("s_out", [128, 8 * NCG], dt.float32, kind="ExternalOutput").ap()

    with tile.TileContext(nc) as tc, ExitStack() as ctx:
        const = ctx.enter_context(tc.tile_pool(name="const", bufs=1))
        stage = ctx.enter_context(tc.tile_pool(name="stage", bufs=3))
        emb = ctx.enter_context(tc.tile_pool(name="emb", bufs=1))
        band = ctx.enter_context(tc.tile_pool(name="band", bufs=1))
        prodp = ctx.enter_context(tc.tile_pool(name="prodp", bufs=2))
        rows = ctx.enter_context(tc.tile_pool(name="rows", bufs=1))
        esc = ctx.enter_context(tc.tile_pool(name="esc2", bufs=2))
        mmp = ctx.enter_context(tc.tile_pool(name="mmp", bufs=2, space="PSUM"))

        # ---- constants -------------------------------------------------
        wt = const.tile([128, KO * DF], dt.float8e4)       # W16 as [k%128, (ko d)]
        nc.sync.dma_start(out=wt[:].rearrange("p (ko d) -> p ko d", ko=KO),
                          in_=w_in[:].rearrange("(ko p) d -> p ko d", p=128))
        wt3 = wt[:].rearrange("p (ko d) -> p ko d", ko=KO)
        b_col = const.tile([128, 2], dt.float32)           # bias per (d%128, dtile)
        nc.sync.dma_start(out=b_col[:], in_=b_in[:].rearrange("(d p) -> p d", p=128))
        ones_f = const.tile([128, 1], dt.float32)
        nc.vector.memset(ones_f[:], 1.0)
        ones = const.tile([128, 1], dt.bfloat16)
        nc.vector.tensor_copy(ones[:], ones_f[:])

        # ---- embedding storage -----------------------------------------
        # fp8 at 16x scale, [128, ksub, cols] layout (ksub = 2 halves of Df)
        e1f = emb.tile([128, 2 * L], dt.float8e4, name="e1f")
        e2f = emb.tile([128, 2 * N], dt.float8e4, name="e2f")
        e1f3 = e1f[:].rearrange("p (k n) -> p k n", k=2)
        e2f3 = e2f[:].rearrange("p (k n) -> p k n", k=2)
        # bf16 copies (16x scale) of e1 and e2 own-batch cols for banded terms
        e1b = emb.tile([128, 2 * L], dt.bfloat16, name="e1b")
        e2b = emb.tile([128, 2 * L], dt.bfloat16, name="e2b")

        def project(src_ap, col0, ncols, dstf, dstride, bf16_dst=None):
            """DoubleRow-project fT cols [col0, col0+ncols) into the fp8 tile
            dstf at flat offset d*dstride+col0 per k-subtile d (16x scale);
            optionally also write the first min(ncols, L) cols as bf16."""
            fst = stage.tile([128, KO * CW], dt.float8e4, tag="fst")
            fst3 = fst[:].rearrange("p (ko n) -> p ko n", ko=KO, n=CW)
            nc.sync.dma_start(
                out=fst3[:, :, 0:ncols],
                in_=src_ap[:, col0:col0 + ncols].rearrange("(ko p) n -> p ko n", p=128))
            for d in range(2):
                ps = mmp.tile([128, ncols], dt.float32, tag="mm", name=f"pj{col0}_{d}")
                for kop in range(KO // 2):
                    for h in range(ncols // 512):
                        nc.tensor.matmul(
                            ps[:, h * 512:(h + 1) * 512],
                            wt3[:, 2 * kop:2 * kop + 2, d * 128:(d + 1) * 128],
                            fst3[:, 2 * kop:2 * kop + 2, h * 512:(h + 1) * 512],
                            start=(kop == 0), stop=(kop == KO // 2 - 1),
                            perf_mode=DR)
                nc.vector.tensor_scalar(
                    dstf[:, d * dstride + col0: d * dstride + col0 + ncols], ps[:],
                    b_col[:, d:d + 1], 0.0, ALU.add, ALU.max)
                if bf16_dst is not None:
                    nb = min(ncols, L)
                    nc.vector.tensor_scalar(
                        bf16_dst[:, d * L:d * L + nb], ps[:, 0:nb],
                        b_col[:, d:d + 1], 0.0, ALU.add, ALU.max)

        # ---- head: f1, f2 chunk 0, main diag ---------------------------
        project(f1t, 0, L, e1f, L, bf16_dst=e1b)
        project(f2t, 0, CW, e2f, N, bf16_dst=e2b)   # own batch = cols 0:L

        def reduce_group(pairs, out_dram, tag):
            """out_dram[j] = sum over pairs (a,b) and d of (a*b)[d, j]; 256x scale."""
            row = rows.tile([1, L], dt.float32, tag=f"row_{tag}")
            for half in range(L // 512):
                rp = mmp.tile([1, 512], dt.float32, tag="mm", name=f"rp_{tag}_{half}")
                for gi, (a_view, b_view) in enumerate(pairs):
                    prod = prodp.tile([128, 512], dt.bfloat16, tag="prod")
                    nc.vector.tensor_tensor(
                        prod[:], a_view[:, half * 512:(half + 1) * 512],
                        b_view[:, half * 512:(half + 1) * 512], ALU.mult)
                    nc.tensor.matmul(rp[:], ones[:], prod[:],
                                     start=(gi == 0), stop=(gi == len(pairs) - 1))
                nc.vector.tensor_copy(row[:, half * 512:(half + 1) * 512], rp[:])
            nc.sync.dma_start(out=out_dram[:].rearrange("(one n) -> one n", one=1), in_=row[:])

        e1bd = [e1b[:, 0:L], e1b[:, L:2 * L]]
        e2bd = [e2b[:, 0:L], e2b[:, L:2 * L]]
        reduce_group(list(zip(e1bd, e2bd)), pos_main, "main")

        # diag bias column layout: [128, 8] with diag[p, a] = pos_main[a*128+p]
        neg_diag = const.tile([128, 8], dt.float32)
        nc.sync.dma_start(out=neg_diag[:], in_=pos_main[:].rearrange("(a p) -> p a", p=128))
        nc.vector.tensor_scalar_mul(neg_diag[:], neg_diag[:], -1.0 / (SC * SC))

        # f2 chunk 1 projects while the diag DMA round-trip completes
        project(f2t, 1 * CW, CW, e2f, N)

        # ---- sim slab + streaming exp-sum, interleaved with projections --
        stot = const.tile([128, 8 * NCG], dt.float32)

        def sim_tile(m, c):
            ps = mmp.tile([128, CW], dt.float32, tag="mm", name=f"sim{m}_{c}")
            for q in range(CW // 512):
                nc.tensor.matmul(
                    ps[:, q * 512:(q + 1) * 512],
                    e1f3[:, :, m * 128:(m + 1) * 128],
                    e2f3[:, :, c * CW + q * 512: c * CW + (q + 1) * 512],
                    start=True, stop=True, perf_mode=DR)
            ex = esc.tile([128, CW], dt.bfloat16, tag="ex")
            nc.scalar.activation(ex[:], ps[:], AF.Exp,
                                 bias=neg_diag[:, m:m + 1], scale=1.0 / (SC * SC),
                                 accum_out=stot[:, m * NCG + c: m * NCG + c + 1])

        # column group 0 first (own chunk already projected); remaining f2
        # chunks interleave between its early tiles so ACT never waits later.
        sim_tile(0, 0)
        sim_tile(1, 0)
        project(f2t, 2 * CW, CW, e2f, N)
        sim_tile(2, 0)
        sim_tile(3, 0)
        project(f2t, 3 * CW, CW, e2f, N)
        for m in range(4, 8):
            sim_tile(m, 0)
        for c in range(1, NCG):
            for m in range(8):
                sim_tile(m, c)

        # ---- banded positive terms -------------------------------------
        # DVE work is emitted after the last projection epilogue; it fills
        # the DVE queue while ACT streams the c1..c3 EXPs.
        def boxsum(src_view, r, tag):
            """Return [128, L] view/tile: out[:, j] = sum_{|d|<=r} src[:, j+d] (clipped)."""
            wdt = 2 * r + 1
            pb = band.tile([128, L + 2 * PAD], dt.bfloat16, name=f"pb_{tag}",
                           tag="pb", bufs=2)
            nc.vector.memzero(pb[:])
            nc.vector.tensor_copy(pb[:, PAD:PAD + L], src_view)
            s = {1: pb}
            for p in (2, 4, 8):
                if wdt >= p:
                    sp = band.tile([128, L + 2 * PAD], dt.bfloat16, name=f"s{p}_{tag}",
                                   tag=f"s{p}")
                    h = p // 2
                    n_valid = L + 2 * PAD - p + 1
                    nc.vector.tensor_tensor(
                        sp[:, :n_valid], s[h][:, :n_valid], s[h][:, h:h + n_valid], ALU.add)
                    s[p] = sp
            terms = _box_terms(wdt)
            t0 = PAD - r
            if len(terms) == 1:
                p0, o0 = terms[0]
                return s[p0][:, t0 + o0: t0 + o0 + L]
            acc = band.tile([128, L], dt.bfloat16, name=f"box_{tag}", tag="box", bufs=6)
            p0, o0 = terms[0]
            p1, o1 = terms[1]
            nc.vector.tensor_tensor(acc[:], s[p0][:, t0 + o0: t0 + o0 + L],
                                    s[p1][:, t0 + o1: t0 + o1 + L], ALU.add)
            for p, o in terms[2:]:
                nc.vector.tensor_tensor(acc[:], acc[:], s[p][:, t0 + o: t0 + o + L], ALU.add)
            return acc[:]

        if r_self > 0:
            bx1 = [boxsum(e1bd[d], r_self, f"s1_{d}") for d in range(2)]
            bx2 = [boxsum(e2bd[d], r_self, f"s2_{d}") for d in range(2)]
            self_pairs = ([(e1bd[d], bx1[d]) for d in range(2)]
                          + [(e2bd[d], bx2[d]) for d in range(2)])
            reduce_group(self_pairs, pos_self, "self")
        else:
            zr = rows.tile([1, L], dt.float32, tag="zr")
            nc.vector.memset(zr[:], 0.0)
            nc.sync.dma_start(out=pos_self[:].rearrange("(one n) -> one n", one=1), in_=zr[:])
        if r_tgt > 0:
            if r_tgt == r_self:
                bxt = bx2                      # identical boxsum, reuse
            else:
                bxt = [boxsum(e2bd[d], r_tgt, f"t_{d}") for d in range(2)]
            reduce_group([(e1bd[d], bxt[d]) for d in range(2)], pos_tgt, "tgt")
        else:
            zr2 = rows.tile([1, L], dt.float32, tag="zr2")
            nc.vector.memset(zr2[:], 0.0)
            nc.sync.dma_start(out=pos_tgt[:].rearrange("(one n) -> one n", one=1), in_=zr2[:])

        nc.sync.dma_start(out=s_out[:], in_=stot[:])

    nc.compile()
    return nc


def kernel(feature1, feature2, W, b, positive_range_self, positive_range_tgt):
    r_self = int(np.asarray(positive_range_self))
    r_tgt = int(np.asarray(positive_range_tgt))
    assert 0 <= r_self <= PAD and 0 <= r_tgt <= PAD

    key = (r_self, r_tgt)
    if key not in _module_cache:
        _module_cache[key] = _build(r_self, r_tgt)
    nc = _module_cache[key]

    in_maps = _make_in_maps(feature1, feature2, W, b)
    res = bass_utils.run_bass_kernel_spmd(nc, in_maps, list(range(N_CORES)))

    # ---- host combine (fp64) ---------------------------------------------
    j = np.arange(L)
    s2 = SC * SC
    loss_terms = []
    for i in range(N_CORES):
        r = res.results[i]
        # S groups: stot[p, m*NCG + c]; token j = m*128 + p; sum over c groups
        S = r["s_out"].astype(np.float64).reshape(128, 8, NCG).sum(axis=2)
        S = S.T.reshape(L)                                   # token j at [j%128, j//128]
        t = np.log(S) - np.log(float(N))                     # negative - diag (diag cancels)
        if r_self > 0:
            cnt = np.minimum(L - 1, j + r_self) - np.maximum(0, j - r_self) + 1.0
            t -= r["pos_self"].astype(np.float64) / s2 / cnt
        if r_tgt > 0:
            cnt = np.minimum(L - 1, j + r_tgt) - np.maximum(0, j - r_tgt) + 1.0
            t -= r["pos_tgt"].astype(np.float64) / s2 / cnt
        loss_terms.append(t)
    loss = np.mean(np.concatenate(loss_terms))
    return np.float32(loss)


def _make_in_maps(feature1, feature2, W, b):
    f8 = ml_dtypes.float8_e4m3fn
    f1 = np.asarray(feature1, dtype=np.float32)
    f2 = np.asarray(feature2, dtype=np.float32)
    Wr = np.ascontiguousarray(np.clip(SC * np.asarray(W, dtype=np.float32),
                                      -240, 240).astype(f8))
    bv = np.ascontiguousarray(SC * np.asarray(b, dtype=np.float32))
    f2t_full = np.clip(f2.reshape(N, DIN).T, -240, 240).astype(f8)   # [DIN, N]
    in_maps = []
    for i in range(N_CORES):
        f1t_i = np.ascontiguousarray(np.clip(f1[i].T, -240, 240).astype(f8))
        f2t_rot = np.ascontiguousarray(np.roll(f2t_full, -i * L, axis=1))
        in_maps.append({"f1t": f1t_i, "f2t": f2t_rot, "w_in": Wr, "b_in": bv})
    return in_maps


# revision 7
# speedup vs baseline: 1.1576x; 1.1576x over previous
"""Contrastive-learning NCE loss on 8 trn2 NeuronCores (Bass/Tile).

Problem (hardcoded shapes): B=8, L=1024, D_in=512, D_feat=256, N=B*L=8192.
  emb_k = relu(feature_k @ W + b)                     [B, L, Df]
  positive = <e1,e2> + banded_diag_mean terms         [N]
  negative = logsumexp(e1 @ e2.T, axis=-1) - log(N)   [N]
  loss = mean(-positive + negative)

Sharding: token dim N split across 8 cores = one batch row each (L == N/8).
Each core computes its [1024, 8192] slab of the similarity matrix against the
full emb_2 (recomputed locally from full feature2). The host rotates feature2
per core so the core's own batch always sits at columns 0:1023 -> the SPMD
program is core-index free.

fp8 pipeline: all matmuls are fp8e4m3 MatmulPerfMode.DoubleRow (K=256/inst,
2 elem/cycle moving stream at full p-state). Host pre-scales W,b by 16 so W
leaves the fp8 subnormal range; embeddings are stored at 16x scale. Every dot
product is 256x true scale: EXP applies scale=1/256, host divides pos_* by
256. Host inputs are pre-swizzled to [128, ko, n] so every DMA line is
contiguous per partition.

The kernel is ACT-bound (exp over the slab = (2048+352)/1.2 + 187 ns per
[128,2048] tile = ~65us/core). Schedule: project f1+chunk0, then stream sim
column group 0 while chunks 1-3 project between sim tiles on the PE. The exp
shift is a CONSTANT K=48 (no per-row diag bias): removes the diag DMA
round-trip from the critical path; safe because max sim ~= 120 -> exp args
<= ~72, inside fp32/bf16 range. The banded terms read bf16 casts of the fp8
embeddings (fp8->bf16 is lossless, so the main positive term matches the sim
diagonal exactly); their DVE work and PE row-sum matmuls are interleaved into
the later sim column groups where both engines have slack. ACT's exp table is
preloaded with a dummy activation during the DMA head.
"""

import numpy as np
import ml_dtypes
from contextlib import ExitStack

import concourse.bass as bass
import concourse.tile as tile
from concourse import bacc, mybir
from concourse import bass_utils

dt = mybir.dt
AF = mybir.ActivationFunctionType
ALU = mybir.AluOpType
DR = mybir.MatmulPerfMode.DoubleRow

N_CORES = 8
B, L, DIN, DF = 8, 1024, 512, 256
N = B * L
KO = DIN // 128     # 4 k-tiles of the projection contraction
PAD = 4             # box-filter padding (max supported positive_range)
LP = L + 2 * PAD    # padded row length for banded box sums
CW = 2048           # column group width: f2 proj chunks AND sim/EXP tiles
NCG = N // CW       # 4 column groups
SC = 16.0           # host pre-scale on W, b; emb stored at 16x
KSHIFT = 48.0       # constant exp shift (max sim ~120 -> args <= ~72)

_module_cache = {}


def _box_terms(w: int):
    """Decompose window width w (odd, <= 2*PAD+1) into power-of-2 segments:
    returns [(pow, offset), ...] s.t. window = concat of segments."""
    terms, off = [], 0
    for p in (8, 4, 2, 1):
        if w >= p:
            terms.append((p, off))
            off += p
            w -= p
    assert w == 0
    return terms


def _build(r_self: int, r_tgt: int):
    nc = bacc.Bacc("TRN2", target_bir_lowering=False, debug=False, num_devices=N_CORES)

    f1d = nc.dram_tensor("f1d", [128, KO, L], dt.float8e4, kind="ExternalInput").ap()
    f2d = nc.dram_tensor("f2d", [128, KO, N], dt.float8e4, kind="ExternalInput").ap()
    w_in = nc.dram_tensor("w_in", [128, KO, DF], dt.float8e4, kind="ExternalInput").ap()
    b_in = nc.dram_tensor("b_in", [128, 2], dt.float32, kind="ExternalInput").ap()

    pos_main = nc.dram_tensor("pos_main", [L], dt.float32, kind="ExternalOutput").ap()
    pos_self = nc.dram_tensor("pos_self", [L], dt.float32, kind="ExternalOutput").ap()
    pos_tgt = nc.dram_tensor("pos_tgt", [L], dt.float32, kind="ExternalOutput").ap()
    s_out = nc.dram_tensor("s_out", [128, 8 * NCG], dt.float32, kind="ExternalOutput").ap()

    with tile.TileContext(nc) as tc, ExitStack() as ctx:
        const = ctx.enter_context(tc.tile_pool(name="const", bufs=1))
        stage = ctx.enter_context(tc.tile_pool(name="stage", bufs=3))
        emb = ctx.enter_context(tc.tile_pool(name="emb", bufs=1))
        band = ctx.enter_context(tc.tile_pool(name="band", bufs=1))
        prodp = ctx.enter_context(tc.tile_pool(name="prodp", bufs=2))
        rows = ctx.enter_context(tc.tile_pool(name="rows", bufs=1))
        esc = ctx.enter_context(tc.tile_pool(name="esc2", bufs=2))
        mmp = ctx.enter_context(tc.tile_pool(name="mmp", bufs=2, space="PSUM"))

        # ---- constants + exp-table preload -----------------------------
        wt = const.tile([128, KO * DF], dt.float8e4)
        wt3 = wt[:].rearrange("p (ko d) -> p ko d", ko=KO)
        nc.sync.dma_start(out=wt3, in_=w_in[:])
        b_col = const.tile([128, 2], dt.float32)
        nc.sync.dma_start(out=b_col[:], in_=b_in[:])
        ones_f = const.tile([128, 1], dt.float32)
        nc.vector.memset(ones_f[:], 1.0)
        ones = const.tile([128, 1], dt.bfloat16)
        nc.vector.tensor_copy(ones[:], ones_f[:])
        dum = const.tile([128, 1], dt.bfloat16)
        nkb = const.tile([128, 1], dt.float32)             # -KSHIFT bias column
        nc.vector.memset(nkb[:], -KSHIFT)
        nc.scalar.activation(dum[:], ones_f[:], AF.Exp, bias=0.0, scale=1.0)

        # ---- embedding storage -----------------------------------------
        # fp8 at 16x scale, [128, ksub, cols] layout (ksub = 2 halves of Df)
        e1f = emb.tile([128, 2 * L], dt.float8e4, name="e1f")
        e2f = emb.tile([128, 2 * N], dt.float8e4, name="e2f")
        e1f3 = e1f[:].rearrange("p (k n) -> p k n", k=2)
        e2f3 = e2f[:].rearrange("p (k n) -> p k n", k=2)

        def project(src3, col0, ncols, dstf, dstride):
            """DoubleRow-project pre-swizzled cols [col0, col0+ncols) into the
            fp8 tile dstf at flat offset d*dstride+col0 per k-subtile d."""
            fst = stage.tile([128, KO * CW], dt.float8e4, tag="fst")
            fst3 = fst[:].rearrange("p (ko n) -> p ko n", ko=KO, n=CW)
            nc.sync.dma_start(out=fst3[:, :, 0:ncols],
                              in_=src3[:, :, col0:col0 + ncols])
            for d in range(2):
                ps = mmp.tile([128, ncols], dt.float32, tag="mm", name=f"pj{col0}_{d}")
                for kop in range(KO // 2):
                    for h in range(ncols // 512):
                        nc.tensor.matmul(
                            ps[:, h * 512:(h + 1) * 512],
                            wt3[:, 2 * kop:2 * kop + 2, d * 128:(d + 1) * 128],
                            fst3[:, 2 * kop:2 * kop + 2, h * 512:(h + 1) * 512],
                            start=(kop == 0), stop=(kop == KO // 2 - 1),
                            perf_mode=DR)
                nc.vector.tensor_scalar(
                    dstf[:, d * dstride + col0: d * dstride + col0 + ncols], ps[:],
                    b_col[:, d:d + 1], 0.0, ALU.add, ALU.max)

        # ---- sim tile: 4 DR matmuls + one EXP with row-sum accumulate ---
        stot = const.tile([128, 8 * NCG], dt.float32)

        def sim_tile(m, c):
            ps = mmp.tile([128, CW], dt.float32, tag="mm", name=f"sim{m}_{c}")
            for q in range(CW // 512):
                nc.tensor.matmul(
                    ps[:, q * 512:(q + 1) * 512],
                    e1f3[:, :, m * 128:(m + 1) * 128],
                    e2f3[:, :, c * CW + q * 512: c * CW + (q + 1) * 512],
                    start=True, stop=True, perf_mode=DR)
            ex = esc.tile([128, CW], dt.bfloat16, tag="ex")
            nc.scalar.activation(ex[:], ps[:], AF.Exp,
                                 bias=nkb[:, 0:1], scale=1.0 / (SC * SC),
                                 accum_out=stot[:, m * NCG + c: m * NCG + c + 1])

        def reduce_group(pairs, out_dram, tag):
            """out_dram[j] = sum over pairs (a,b) and d of (a*b)[d, j]; 256x scale."""
            row = rows.tile([1, L], dt.float32, tag=f"row_{tag}")
            for half in range(L // 512):
                rp = mmp.tile([1, 512], dt.float32, tag="mm", name=f"rp_{tag}_{half}")
                for gi, (a_view, b_view) in enumerate(pairs):
                    prod = prodp.tile([128, 512], dt.bfloat16, tag="prod")
                    nc.vector.tensor_tensor(
                        prod[:], a_view[:, half * 512:(half + 1) * 512],
                        b_view[:, half * 512:(half + 1) * 512], ALU.mult)
                    nc.tensor.matmul(rp[:], ones[:], prod[:],
                                     start=(gi == 0), stop=(gi == len(pairs) - 1))
                nc.vector.tensor_copy(row[:, half * 512:(half + 1) * 512], rp[:])
            nc.sync.dma_start(out=out_dram[:].rearrange("(one n) -> one n", one=1), in_=row[:])

        # ---- schedule ---------------------------------------------------
        project(f1d, 0, L, e1f, L)
        project(f2d, 0 * CW, CW, e2f, N)
        for m in range(4):
            sim_tile(m, 0)
        project(f2d, 1 * CW, CW, e2f, N)
        for m in range(4, 8):
            sim_tile(m, 0)
        project(f2d, 2 * CW, CW, e2f, N)
        for m in range(4):
            sim_tile(m, 1)
        project(f2d, 3 * CW, CW, e2f, N)
        for m in range(4, 8):
            sim_tile(m, 1)

        # bf16 casts (lossless from fp8) into padded tiles + banded boxsums;
        # pure DVE work that drains while ACT streams the c=1..3 EXPs.
        e1bp = band.tile([128, 2 * LP], dt.bfloat16, name="e1bp")
        e2bp = band.tile([128, 2 * LP], dt.bfloat16, name="e2bp")
        nc.vector.memzero(e1bp[:])
        nc.vector.memzero(e2bp[:])
        for d in range(2):
            nc.vector.tensor_copy(e1bp[:, d * LP + PAD: d * LP + PAD + L],
                                  e1f[:, d * L: (d + 1) * L])
            nc.vector.tensor_copy(e2bp[:, d * LP + PAD: d * LP + PAD + L],
                                  e2f[:, d * N: d * N + L])
        e1bd = [e1bp[:, d * LP + PAD: d * LP + PAD + L] for d in range(2)]
        e2bd = [e2bp[:, d * LP + PAD: d * LP + PAD + L] for d in range(2)]

        def boxsum(pb, r, tag):
            """pb: [128, LP] padded view (zeros in pads). Returns [128, L]
            view/tile: out[:, j] = sum_{|dd|<=r} pb[:, j+PAD+dd] (clipped)."""
            wdt = 2 * r + 1
            s = {1: pb}
            for p in (2, 4, 8):
                if wdt >= p:
                    sp = band.tile([128, LP], dt.bfloat16, name=f"s{p}_{tag}",
                                   tag=f"s{p}", bufs=2)
                    h = p // 2
                    n_valid = LP - p + 1
                    nc.vector.tensor_tensor(
                        sp[:, :n_valid], s[h][:, :n_valid], s[h][:, h:h + n_valid], ALU.add)
                    s[p] = sp
            terms = _box_terms(wdt)
            t0 = PAD - r
            if len(terms) == 1:
                p0, o0 = terms[0]
                return s[p0][:, t0 + o0: t0 + o0 + L]
            acc = band.tile([128, L], dt.bfloat16, name=f"box_{tag}", tag="box", bufs=6)
            p0, o0 = terms[0]
            p1, o1 = terms[1]
            nc.vector.tensor_tensor(acc[:], s[p0][:, t0 + o0: t0 + o0 + L],
                                    s[p1][:, t0 + o1: t0 + o1 + L], ALU.add)
            for p, o in terms[2:]:
                nc.vector.tensor_tensor(acc[:], acc[:], s[p][:, t0 + o: t0 + o + L], ALU.add)
            return acc[:]

        if r_self > 0:
            bx1 = [boxsum(e1bp[:, d * LP: (d + 1) * LP], r_self, f"s1_{d}") for d in range(2)]
            bx2 = [boxsum(e2bp[:, d * LP: (d + 1) * LP], r_self, f"s2_{d}") for d in range(2)]
        if r_tgt > 0:
            if r_tgt == r_self and r_self > 0:
                bxt = bx2                      # identical boxsum, reuse
            else:
                bxt = [boxsum(e2bp[:, d * LP: (d + 1) * LP], r_tgt, f"t_{d}") for d in range(2)]

        # sim c=2/c=3 with the banded row-sum matmuls interleaved (PE and
        # ACT both have slack here; psum pool slots alternate naturally)
        sim_tile(0, 2)
        sim_tile(1, 2)
        reduce_group(list(zip(e1bd, e2bd)), pos_main, "main")
        sim_tile(2, 2)
        sim_tile(3, 2)
        if r_self > 0:
            reduce_group([(e1bd[d], bx1[d]) for d in range(2)]
                         + [(e2bd[d], bx2[d]) for d in range(2)], pos_self, "self")
        else:
            zr = rows.tile([1, L], dt.float32, tag="zr")
            nc.vector.memset(zr[:], 0.0)
            nc.sync.dma_start(out=pos_self[:].rearrange("(one n) -> one n", one=1), in_=zr[:])
        for m in range(4, 8):
            sim_tile(m, 2)
        if r_tgt > 0:
            reduce_group([(e1bd[d], bxt[d]) for d in range(2)], pos_tgt, "tgt")
        else:
            zr2 = rows.tile([1, L], dt.float32, tag="zr2")
            nc.vector.memset(zr2[:], 0.0)
            nc.sync.dma_start(out=pos_tgt[:].rearrange("(one n) -> one n", one=1), in_=zr2[:])
        for m in range(8):
            sim_tile(m, 3)

        nc.sync.dma_start(out=s_out[:], in_=stot[:])

    nc.compile()
    return nc


def kernel(feature1, feature2, W, b, positive_range_self, positive_range_tgt):
    r_self = int(np.asarray(positive_range_self))
    r_tgt = int(np.asarray(positive_range_tgt))
    assert 0 <= r_self <= PAD and 0 <= r_tgt <= PAD

    key = (r_self, r_tgt)
    if key not in _module_cache:
        _module_cache[key] = _build(r_self, r_tgt)
    nc = _module_cache[key]

    in_maps = _make_in_maps(feature1, feature2, W, b)
    res = bass_utils.run_bass_kernel_spmd(nc, in_maps, list(range(N_CORES)))

    # ---- host combine (fp64) ---------------------------------------------
    j = np.arange(L)
    s2 = SC * SC
    loss_terms = []
    for i in range(N_CORES):
        r = res.results[i]
        # S groups: stot[p, m*NCG + c]; token j = m*128 + p; sum over c groups
        S = r["s_out"].astype(np.float64).reshape(128, 8, NCG).sum(axis=2)
        S = S.T.reshape(L)                                   # token j at [j%128, j//128]
        t = KSHIFT + np.log(S) - np.log(float(N))            # negative term
        t -= r["pos_main"].astype(np.float64) / s2
        if r_self > 0:
            cnt = np.minimum(L - 1, j + r_self) - np.maximum(0, j - r_self) + 1.0
            t -= r["pos_self"].astype(np.float64) / s2 / cnt
        if r_tgt > 0:
            cnt = np.minimum(L - 1, j + r_tgt) - np.maximum(0, j - r_tgt) + 1.0
            t -= r["pos_tgt"].astype(np.float64) / s2 / cnt
        loss_terms.append(t)
    loss = np.mean(np.concatenate(loss_terms))
    return np.float32(loss)


def _swizzle(a_t):
    """[DIN, n] -> [128, KO, n] with row k at [k % 128, k // 128]."""
    n = a_t.shape[1]
    return np.ascontiguousarray(a_t.reshape(KO, 128, n).transpose(1, 0, 2))


def _make_in_maps(feature1, feature2, W, b):
    f8 = ml_dtypes.float8_e4m3fn
    f1 = np.asarray(feature1, dtype=np.float32)
    f2 = np.asarray(feature2, dtype=np.float32)
    Wr = _swizzle(np.clip(SC * np.asarray(W, dtype=np.float32), -240, 240).astype(f8))
    bv = np.ascontiguousarray(
        (SC * np.asarray(b, dtype=np.float32)).reshape(2, 128).T)
    f2t_full = np.clip(f2.reshape(N, DIN).T, -240, 240).astype(f8)   # [DIN, N]
    in_maps = []
    for i in range(N_CORES):
        f1s = _swizzle(np.clip(f1[i].T, -240, 240).astype(f8))
        f2s = _swizzle(np.roll(f2t_full, -i * L, axis=1))
        in_maps.append({"f1d": f1s, "f2d": f2s, "w_in": Wr, "b_in": bv})
    return in_maps
